# revision 1
# baseline (speedup 1.0000x reference)
"""CrystalGraphEncoder (2x TransformerConv + 2x GATConv + LN + mean-pool + MLP)
as a Bass/Tile kernel on 8 Trainium2 NeuronCores.

Strategy: shard destination nodes across cores (edges sorted by dst). Per
layer: sharded dense matmuls -> fp8 gather tables (kv / q / hh+a_s packed)
-> AllGather in two halves into Shared-HBM tables (split by source node so
the second half overlaps edge compute) -> bulk dma_gather of per-edge rows,
two dst-blocks per call -> DVE edge math (fp8 inputs, f16 products) ->
one-hot (fp8) scatter matmuls into PSUM -> pair-batched normalize + skip +
residual + LN on-chip. The dense phase of layer L+1 is interleaved into the
edge phase of layer L so its AllGather halves are emitted as early as their
bounce rows are ready. Mean-pool via one-hot matmul + AllReduce; final MLP
replicated.
"""
import numpy as np
import ml_dtypes

import concourse.bacc as bacc
import concourse.tile as tile
from concourse import bass, mybir
from concourse import bass_utils
from concourse.masks import make_identity

F16 = mybir.dt.float16
F32 = mybir.dt.float32
F8 = mybir.dt.float8e4
I16 = mybir.dt.int16
NP_F16 = np.float16
NP_F8 = ml_dtypes.float8_e4m3
P = 128

# problem constants (from the reference model)
N_NODES = 20000
IN_DIM = 92
HID = 256
OUT_DIM = 128
HEADS = 8
HDIM = 32
N_GRAPHS = 128
LN_EPS = 1e-5
C = 8  # cores
AF = mybir.ActivationFunctionType


def _wrap_idxs(idx):
    """[n] int -> [128, n//16] int16 dma_gather index layout (16-partition wrap,
    replicated for the 8 Q7 cores)."""
    n = len(idx)
    assert n % 16 == 0
    w = idx.reshape(n // 16, 16).T.astype(np.int16)
    return np.ascontiguousarray(np.tile(w, (8, 1)))


def _edge_struct(src_half_row, src_half, dst_local, dst_core, NB, pad_q):
    """Per-core gather/scatter arrays for one edge set, split by source half
    so the AllGather can be chunked into two overlappable halves.

    Per dst block: slots are [NTA j-tiles of half-0-src edges | NTB j-tiles of
    half-1-src edges]. kv idx arrays are stored per half (block-major) so a
    block-pair's half gather is one contiguous slice; q idx is stored
    pair-major in tile order [A(b) A(b+1) B(b) B(b+1)] to match the gather
    output tile layout.
    """
    blk = dst_local // P
    slot = dst_local % P
    key = (dst_core * NB + blk) * 2 + src_half
    order = np.argsort(key, kind="stable")
    src_s = src_half_row[order]
    slot_s = slot[order]
    loc_s = dst_local[order]
    counts = np.bincount(key, minlength=C * NB * 2)
    NTA = int(np.ceil(counts[0::2].max() / P))
    NTB = int(np.ceil(counts[1::2].max() / P))
    starts = np.concatenate([[0], np.cumsum(counts)])

    per_core = []
    for c in range(C):
        kvA = np.zeros((NB, NTA * P), dtype=np.int64)  # pad -> row 0
        kvB = np.zeros((NB, NTB * P), dtype=np.int64)
        q_idx = np.full((NB, (NTA + NTB) * P), pad_q, dtype=np.int64)
        S = np.zeros((NB, P, (NTA + NTB) * P), dtype=NP_F8)
        for b in range(NB):
            for half, kv_arr, base in ((0, kvA, 0), (1, kvB, NTA * P)):
                k = (c * NB + b) * 2 + half
                s, e = starts[k], starts[k + 1]
                n = e - s
                pos = np.arange(n)
                kv_arr[b, pos] = src_s[s:e]
                q_idx[b, base + pos] = loc_s[s:e]
                jj = pos // P
                pp = pos % P
                S[b, pp, (base // P + jj) * P + slot_s[s:e]] = 1.0
        # q idx pair-major in tile order [A(b), A(b+1), B(b), B(b+1)]
        qp = []
        for bp in range(0, NB, 2):
            qp.append(q_idx[bp : bp + 2, 0 : NTA * P].reshape(-1))
            qp.append(q_idx[bp : bp + 2, NTA * P :].reshape(-1))
        per_core.append(
            dict(
                kv_idxA=_wrap_idxs(kvA.reshape(-1)),
                kv_idxB=_wrap_idxs(kvB.reshape(-1)),
                q_idx=_wrap_idxs(np.concatenate(qp)),
                S=np.ascontiguousarray(S.reshape(NB * P, (NTA + NTB) * P)),
            )
        )
    return NTA, NTB, per_core


def host_prep(inputs):
    """Split + pad + sort everything on the host. Returns (meta, in_maps)."""
    x = np.asarray(inputs["x"], np.float32)
    ei = np.asarray(inputs["edge_index"], np.int64)
    batch = np.asarray(inputs["batch"], np.int64)
    N = x.shape[0]
    RPC = (N + C - 1) // C
    NB = (RPC + P - 1) // P
    NPC = NB * P

    core_of = np.minimum(np.arange(N) // RPC, C - 1)
    local_of = np.arange(N) - core_of * RPC
    HNPC = NPC // 2
    half_of = local_of // HNPC  # which AllGather half this node's row is in
    # row within the half table [C * HNPC, width]
    ghrow = core_of * HNPC + (local_of % HNPC)

    # Global-table pad edges point at row 0: their scatter coefficient is 0,
    # and the local-side (q/ad) pad rows are zero, so they contribute nothing.
    # (The global tables are Shared-DRAM AllGather outputs, which allow only
    # one writer instruction — no dedicated zero row can be written to them.)
    PAD_Q = NPC  # zero row of local tables

    src, dst = ei[0], ei[1]
    NTTA, NTTB, tconv = _edge_struct(
        ghrow[src], half_of[src], local_of[dst], core_of[dst], NB, PAD_Q
    )
    sl = np.arange(N, dtype=np.int64)
    src_g = np.concatenate([src, sl])
    dst_g = np.concatenate([dst, sl])
    NTGA, NTGB, gat = _edge_struct(
        ghrow[src_g], half_of[src_g], local_of[dst_g], core_of[dst_g], NB, PAD_Q
    )

    cnt = np.bincount(batch, minlength=N_GRAPHS).astype(np.float32)
    invcnt = (1.0 / np.maximum(cnt, 1.0)).reshape(N_GRAPHS, 1)

    def f16(a):
        return np.asarray(a, np.float32).astype(NP_F16)

    def pack_k(w):  # [K, N] -> [128, K//128 * N] (chunk-major)
        w = np.asarray(w, np.float32)
        K, Nc = w.shape
        assert K % P == 0
        return np.ascontiguousarray(
            w.reshape(K // P, P, Nc).transpose(1, 0, 2).reshape(P, -1)
        ).astype(NP_F16)

    wdict = dict(
        win=f16(inputs["Win"]),
        b_in=f16(np.asarray(inputs["b_in"]).reshape(1, HID)),
        w1=pack_k(inputs["W1"]),
        b1=f16(np.asarray(inputs["b1"]).reshape(1, 2 * HID)),
        w2=pack_k(inputs["W2"]),
        b2=f16(np.asarray(inputs["b2"]).reshape(1, OUT_DIM)),
        invcnt=invcnt.astype(np.float32),
        zeros_row=np.zeros((1, 2 * HID), NP_F16),
        zeros_row8=np.zeros((1, 2 * HID), NP_F8),
    )
    for t in range(2):
        wkv = np.concatenate(
            [np.asarray(inputs["Wk"][t]), np.asarray(inputs["Wv"][t])], axis=1
        )
        bkv = np.concatenate(
            [np.asarray(inputs["bk"][t]), np.asarray(inputs["bv"][t])]
        )
        wdict[f"wkv{t}"] = pack_k(wkv)
        wdict[f"bkv{t}"] = f16(bkv.reshape(1, 2 * HID))
        wdict[f"wq{t}"] = pack_k(inputs["Wq"][t])
        wdict[f"bq{t}"] = f16(np.asarray(inputs["bq"][t]).reshape(1, HID))
        wdict[f"wsk{t}"] = pack_k(
            np.asarray(inputs["Wskip"][t], np.float64) + np.eye(HID)
        )
        wdict[f"bsk{t}"] = f16(np.asarray(inputs["bskip"][t]).reshape(1, HID))
        wdict[f"wg{t}"] = pack_k(inputs["Wg"][t])
        wdict[f"bg{t}"] = f16(np.asarray(inputs["bg"][t]).reshape(1, HID))
        wdict[f"atts{t}"] = np.ascontiguousarray(
            np.broadcast_to(
                f16(np.asarray(inputs["att_src"][t]).reshape(1, HID)), (P, HID)
            )
        )
        wdict[f"attd{t}"] = np.ascontiguousarray(
            np.broadcast_to(
                f16(np.asarray(inputs["att_dst"][t]).reshape(1, HID)), (P, HID)
            )
        )

    ln_g = np.asarray(inputs["ln_g"], np.float32)
    ln_b = np.asarray(inputs["ln_b"], np.float32)
    ln_trivial = bool(np.all(ln_g == 1.0) and np.all(ln_b == 0.0))
    if not ln_trivial:
        for i in range(4):
            wdict[f"lng{i}"] = np.ascontiguousarray(
                np.broadcast_to(ln_g[i].reshape(1, HID).astype(NP_F16), (P, HID))
            )
            wdict[f"lnb{i}"] = np.ascontiguousarray(
                np.broadcast_to(ln_b[i].reshape(1, HID).astype(NP_F16), (P, HID))
            )

    in_maps = []
    for c in range(C):
        m = dict(wdict)
        lo, hi = c * RPC, min((c + 1) * RPC, N)
        xT = np.zeros((IN_DIM, NPC), np.float32)
        xT[:, 0 : hi - lo] = x[lo:hi].T
        m["xT"] = xT.astype(NP_F16)
        m["kvidxA"] = tconv[c]["kv_idxA"]
        m["kvidxB"] = tconv[c]["kv_idxB"]
        m["qidx"] = tconv[c]["q_idx"]
        m["S_t"] = tconv[c]["S"]
        m["gatidxA"] = gat[c]["kv_idxA"]
        m["gatidxB"] = gat[c]["kv_idxB"]
        m["adidx"] = gat[c]["q_idx"]
        m["S_g"] = gat[c]["S"]
        Sp = np.zeros((NB, P, N_GRAPHS), dtype=NP_F8)
        ns = hi - lo
        bb = np.arange(ns) // P
        pp = np.arange(ns) % P
        Sp[bb, pp, batch[lo:hi]] = 1.0
        m["S_p"] = np.ascontiguousarray(Sp.reshape(NB * P, N_GRAPHS))
        in_maps.append(m)

    meta = dict(
        NB=NB, NPC=NPC, NTTA=NTTA, NTTB=NTTB, NTGA=NTGA, NTGB=NTGB,
        ln_trivial=ln_trivial,
    )
    return meta, in_maps


def build_program(meta):
    NB = meta["NB"]
    NPC = meta["NPC"]
    NTTA, NTTB = meta["NTTA"], meta["NTTB"]
    NTGA, NTGB = meta["NTGA"], meta["NTGB"]
    ln_trivial = meta["ln_trivial"]
    HNPC = NPC // 2
    HTABN = C * HNPC
    LOCN = NPC + 1
    NTT = NTTA + NTTB  # j-tiles per block
    NTG = NTGA + NTGB
    NTMAX = max(NTT, NTG)
    NPAIR = NB // 2

    nc = bacc.Bacc("TRN2", target_bir_lowering=False, debug=False, num_devices=C)

    def di(name, shape, dt):
        return nc.dram_tensor(name, shape, dt, kind="ExternalInput")

    xT_d = di("xT", [IN_DIM, NPC], F16)
    kvidxA_d = di("kvidxA", [P, NB * NTTA * 8], I16)
    kvidxB_d = di("kvidxB", [P, NB * NTTB * 8], I16)
    qidx_d = di("qidx", [P, NB * NTT * 8], I16)
    St_d = di("S_t", [NB * P, NTT * P], F8)
    gatidxA_d = di("gatidxA", [P, NB * NTGA * 8], I16)
    gatidxB_d = di("gatidxB", [P, NB * NTGB * 8], I16)
    adidx_d = di("adidx", [P, NB * NTG * 8], I16)
    Sg_d = di("S_g", [NB * P, NTG * P], F8)
    Sp_d = di("S_p", [NB * P, N_GRAPHS], F8)
    invcnt_d = di("invcnt", [N_GRAPHS, 1], F32)
    zeros_d = di("zeros_row", [1, 2 * HID], F16)
    zeros8_d = di("zeros_row8", [1, 2 * HID], F8)
    win_d = di("win", [IN_DIM, HID], F16)
    bin_d = di("b_in", [1, HID], F16)
    w1_d = di("w1", [P, 2 * 2 * HID], F16)
    b1_d = di("b1", [1, 2 * HID], F16)
    w2_d = di("w2", [P, 4 * OUT_DIM], F16)
    b2_d = di("b2", [1, OUT_DIM], F16)
    wd = {}
    for t in range(2):
        wd[f"wkv{t}"] = di(f"wkv{t}", [P, 2 * 2 * HID], F16)
        wd[f"bkv{t}"] = di(f"bkv{t}", [1, 2 * HID], F16)
        wd[f"wq{t}"] = di(f"wq{t}", [P, 2 * HID], F16)
        wd[f"bq{t}"] = di(f"bq{t}", [1, HID], F16)
        wd[f"wsk{t}"] = di(f"wsk{t}", [P, 2 * HID], F16)
        wd[f"bsk{t}"] = di(f"bsk{t}", [1, HID], F16)
        wd[f"wg{t}"] = di(f"wg{t}", [P, 2 * HID], F16)
        wd[f"bg{t}"] = di(f"bg{t}", [1, HID], F16)
        wd[f"atts{t}"] = di(f"atts{t}", [P, HID], F16)
        wd[f"attd{t}"] = di(f"attd{t}", [P, HID], F16)
    if not ln_trivial:
        for i in range(4):
            wd[f"lng{i}"] = di(f"lng{i}", [P, HID], F16)
            wd[f"lnb{i}"] = di(f"lnb{i}", [P, HID], F16)

    out_d = nc.dram_tensor("out", [N_GRAPHS, OUT_DIM], F32, kind="ExternalOutput")

    h_all = nc.alloc_sbuf_tensor("h_all", [P, NB * HID], F16)
    hT_all = nc.alloc_sbuf_tensor("hT_all", [P, 2 * NPC], F16)
    xT_sb = nc.alloc_sbuf_tensor("xT_sb", [IN_DIM, NPC], F16)
    # per-layer index buffers, reloaded at each layer start
    idxA_sb = nc.alloc_sbuf_tensor("idxA_sb", [P, NB * max(NTTA, NTGA) * 8], I16)
    idxB_sb = nc.alloc_sbuf_tensor("idxB_sb", [P, NB * max(NTTB, NTGB) * 8], I16)
    qad_sb = nc.alloc_sbuf_tensor("qad_sb", [P, NB * NTMAX * 8], I16)

    SQ32 = 1.0 / float(np.sqrt(HDIM))

    with tile.TileContext(nc) as tc:
        with (
            tc.tile_pool(name="wpool", bufs=1) as wp,
            tc.tile_pool(name="spool", bufs=2) as sp,
            tc.tile_pool(name="gpool", bufs=2) as gp,
            tc.tile_pool(name="psA", bufs=2, space="PSUM") as psA,
            tc.tile_pool(name="psB", bufs=2, space="PSUM") as psB,
            tc.tile_pool(name="psT", bufs=1, space="PSUM") as psT,
            tc.tile_pool(name="psC", bufs=1, space="PSUM") as psC,
            tc.tile_pool(name="dram", bufs=1, space="DRAM") as dp,
        ):
            nc.sync.dma_start(xT_sb.ap(), xT_d.ap())

            def load_layer_idx(is_t):
                """Refill the shared index buffers for this layer type."""
                if is_t:
                    nc.sync.dma_start(
                        idxA_sb.ap()[:, 0 : NB * NTTA * 8], kvidxA_d.ap()
                    )
                    nc.sync.dma_start(
                        idxB_sb.ap()[:, 0 : NB * NTTB * 8], kvidxB_d.ap()
                    )
                    nc.sync.dma_start(qad_sb.ap()[:, 0 : NB * NTT * 8], qidx_d.ap())
                else:
                    nc.sync.dma_start(
                        idxA_sb.ap()[:, 0 : NB * NTGA * 8], gatidxA_d.ap()
                    )
                    nc.sync.dma_start(
                        idxB_sb.ap()[:, 0 : NB * NTGB * 8], gatidxB_d.ap()
                    )
                    nc.sync.dma_start(qad_sb.ap()[:, 0 : NB * NTG * 8], adidx_d.ap())

            ident = wp.tile([P, P], F16, tag="ident")
            make_identity(nc, ident[:])
            ones1 = wp.tile([1, P], F16, tag="ones1")
            nc.vector.memset(ones1[:], 1.0)
            eps_t = wp.tile([P, 1], F32, tag="eps")
            nc.vector.memset(eps_t[:], LN_EPS)

            def load_w(d, shape, tag, dt=F16):
                t = wp.tile(shape, dt, tag=tag)
                nc.sync.dma_start(t[:], d.ap())
                return t

            win_t = load_w(win_d, [IN_DIM, HID], "win")
            bin_t = load_w(bin_d, [1, HID], "b_in")
            w1_t = load_w(w1_d, [P, 2 * 2 * HID], "w1")
            b1_t = load_w(b1_d, [1, 2 * HID], "b1")
            w2_t = load_w(w2_d, [P, 4 * OUT_DIM], "w2")
            b2_t = load_w(b2_d, [1, OUT_DIM], "b2")
            wt = {}
            for t in range(2):
                for nm, sh in [
                    (f"wkv{t}", [P, 2 * 2 * HID]),
                    (f"bkv{t}", [1, 2 * HID]),
                    (f"wq{t}", [P, 2 * HID]),
                    (f"bq{t}", [1, HID]),
                    (f"wsk{t}", [P, 2 * HID]),
                    (f"bsk{t}", [1, HID]),
                    (f"wg{t}", [P, 2 * HID]),
                    (f"bg{t}", [1, HID]),
                    (f"atts{t}", [P, HID]),
                    (f"attd{t}", [P, HID]),
                ]:
                    wt[nm] = load_w(wd[nm], sh, nm)
            if not ln_trivial:
                for i in range(4):
                    wt[f"lng{i}"] = load_w(wd[f"lng{i}"], [P, HID], f"lng{i}")
                    wt[f"lnb{i}"] = load_w(wd[f"lnb{i}"], [P, HID], f"lnb{i}")
            invcnt_t = load_w(invcnt_d, [N_GRAPHS, 1], "invcnt", F32)
            zrow_t = load_w(zeros_d, [1, 2 * HID], "zrow")
            zrow8_t = load_w(zeros8_d, [1, 2 * HID], "zrow8", F8)
            Sp_sb = []
            for b in range(NB):
                spt = wp.tile([P, N_GRAPHS], F8, tag=f"S_p{b}")
                nc.sync.dma_start(spt[:], Sp_d.ap()[b * P : (b + 1) * P, :])
                Sp_sb.append(spt)

            def mm_dense(psum, lhsT0, lhsT1, w_tile, ncols, bias_tile):
                nc.tensor.matmul(
                    psum, lhsT=lhsT0, rhs=w_tile[:, 0:ncols], start=True, stop=False
                )
                nc.tensor.matmul(
                    psum, lhsT=lhsT1, rhs=w_tile[:, ncols : 2 * ncols],
                    start=False, stop=False,
                )
                nc.tensor.matmul(
                    psum, lhsT=ones1[:], rhs=bias_tile[:, 0:ncols],
                    start=False, stop=True,
                )

            def hT_slices(b):
                l0 = hT_all.ap()[:, 0 * NPC + b * P : 0 * NPC + (b + 1) * P]
                l1 = hT_all.ap()[:, 1 * NPC + b * P : 1 * NPC + (b + 1) * P]
                return l0, l1

            def store_hT(b):
                """h_all (f16) block b -> transposed copies in hT_all."""
                hsl = h_all.ap()[:, b * HID : (b + 1) * HID]
                for f in range(2):
                    ptp = psT.tile([P, P], F16, space="PSUM", tag="ptp")
                    nc.tensor.transpose(ptp[:], hsl[:, f * P : (f + 1) * P], ident[:])
                    nc.scalar.activation(
                        hT_all.ap()[:, f * NPC + b * P : f * NPC + (b + 1) * P],
                        ptp[:],
                        AF.Copy,
                    )

            # phase 0: h0 = x @ Win + b_in
            for b in range(NB):
                ps = psA.tile([P, 2 * HID], F32, space="PSUM", tag="ps_dense")
                nc.tensor.matmul(
                    ps[:, 0:HID], lhsT=xT_sb.ap()[:, b * P : (b + 1) * P],
                    rhs=win_t[:], start=True, stop=False,
                )
                nc.tensor.matmul(
                    ps[:, 0:HID], lhsT=ones1[:], rhs=bin_t[:], start=False, stop=True
                )
                nc.scalar.activation(
                    h_all.ap()[:, b * HID : (b + 1) * HID], ps[:, 0:HID], AF.Copy
                )
                store_hT(b)

            # ---- tables: per-layer, per-half fp8 AllGather outputs ----
            # Each AllGather half is a separate Shared tensor (single-writer
            # rule) so the second half can fly while edges of the first half
            # are being processed. GAT rows are hh-only fp8 (a_s is
            # recomputed per edge on-chip).
            kv_tabs = [
                [
                    dp.tile(
                        [HTABN, 2 * HID], F8, tag=f"kv_tab{t}{h}",
                        name=f"kv_tab{t}{h}", addr_space="Shared",
                    )
                    for h in range(2)
                ]
                for t in range(2)
            ]
            GATW = 256  # f16 cols: 0:128 hh as fp8 (bitcast), 128:136 a_s
            gat_tabs = [
                [
                    dp.tile(
                        [HTABN, GATW], F16, tag=f"gat_tab{t}{h}",
                        name=f"gat_tab{t}{h}", addr_space="Shared",
                    )
                    for h in range(2)
                ]
                for t in range(2)
            ]
            q_tab = dp.tile([LOCN, HID], F8, tag="q_tab")
            ad_tab = dp.tile([LOCN, P], F16, tag="ad_tab")
            kv_bnc = dp.tile([NPC, 2 * HID], F8, tag="kv_bnc")
            gat_bnc = dp.tile([NPC, GATW], F16, tag="gat_bnc")
            pool_in = dp.tile([N_GRAPHS, HID], F32, tag="pool_in")
            pool_out = dp.tile([N_GRAPHS, HID], F32, tag="pool_out")

            psum_pool = psC.tile([N_GRAPHS, HID], F32, space="PSUM", tag="ps_pool")

            # ad_tab rows are gathered 128-wide but only cols 0:HEADS are
            # written per layer — zero the tail once.
            zpad = wp.tile([P, GATW], F16, tag="zpad")
            nc.vector.memset(zpad[:], 0.0)
            for b in range(NB):
                nc.sync.dma_start(
                    ad_tab[b * P : (b + 1) * P, HEADS:P], zpad[:, 0 : P - HEADS]
                )
                nc.sync.dma_start(
                    gat_bnc[b * P : (b + 1) * P, HID // 2 + HEADS : GATW],
                    zpad[:, 0 : GATW - HID // 2 - HEADS],
                )

            def dense_pair(layer, bp):
                """Dense projections for blocks (bp, bp+1) of `layer` into the
                bounce buffers / local tables."""
                is_t = layer % 2 == 0
                t = layer // 2
                nb2 = min(2, NB - bp)
                prow = (
                    lambda tab, c0, c1: tab[bp * P : (bp + nb2) * P, c0:c1]
                    .rearrange("(i p) f -> p i f", i=nb2)
                )
                if is_t:
                    kv8p = sp.tile([P, 2, 2 * HID], F8, tag="kv8")
                    q8p = sp.tile([P, 2, HID], F8, tag="q8")
                else:
                    hh8p = sp.tile([P, 2, HID], F8, tag="hh8")
                    a16p = sp.tile([P, 2, HEADS], F16, tag="a16")
                    as16p = sp.tile([P, 2, HEADS], F16, tag="as16")
                for i in range(nb2):
                    b = bp + i
                    l0, l1 = hT_slices(b)
                    if is_t:
                        ps = psA.tile([P, 2 * HID], F32, space="PSUM", tag="ps_dense")
                        mm_dense(ps[:], l0, l1, wt[f"wkv{t}"], 2 * HID, wt[f"bkv{t}"])
                        nc.scalar.activation(kv8p[:, i, :], ps[:], AF.Copy)
                        ps2 = psB.tile([P, 2 * HID], F32, space="PSUM", tag="ps_b")
                        mm_dense(
                            ps2[:, 0:HID], l0, l1, wt[f"wq{t}"], HID, wt[f"bq{t}"]
                        )
                        nc.scalar.activation(q8p[:, i, :], ps2[:, 0:HID], AF.Copy)
                    else:
                        ps = psA.tile([P, 2 * HID], F32, space="PSUM", tag="ps_dense")
                        mm_dense(
                            ps[:, 0:HID], l0, l1, wt[f"wg{t}"], HID, wt[f"bg{t}"]
                        )
                        hh16 = sp.tile([P, HID], F16, tag="hh16")
                        nc.scalar.activation(hh16[:], ps[:, 0:HID], AF.Copy)
                        nc.scalar.activation(hh8p[:, i, :], ps[:, 0:HID], AF.Copy)
                        for which, wnm in ((0, f"atts{t}"), (1, f"attd{t}")):
                            proda = sp.tile([P, HID], F16, tag="prodA")
                            nc.vector.tensor_tensor(
                                out=proda[:], in0=hh16[:], in1=wt[wnm][:],
                                op=mybir.AluOpType.mult,
                            )
                            asum = sp.tile([P, HEADS], F32, tag="asum")
                            nc.vector.tensor_reduce(
                                out=asum[:],
                                in_=proda[:].rearrange("p (h w) -> p h w", h=HEADS),
                                axis=mybir.AxisListType.X,
                                op=mybir.AluOpType.add,
                            )
                            dst = (as16p if which == 0 else a16p)[:, i, :]
                            nc.scalar.activation(dst, asum[:], AF.Copy)
                if is_t:
                    nc.sync.dma_start(prow(kv_bnc, 0, 2 * HID), kv8p[:, 0:nb2, :])
                    nc.sync.dma_start(prow(q_tab, 0, HID), q8p[:, 0:nb2, :])
                else:
                    nc.sync.dma_start(
                        prow(gat_bnc, 0, HID // 2), hh8p[:, 0:nb2, :].bitcast(F16)
                    )
                    nc.sync.dma_start(
                        prow(gat_bnc, HID // 2, HID // 2 + HEADS), as16p[:, 0:nb2, :]
                    )
                    nc.sync.dma_start(prow(ad_tab, 0, HEADS), a16p[:, 0:nb2, :])

            def emit_ag(layer, half):
                """AllGather one half of this layer's table."""
                is_t = layer % 2 == 0
                t = layer // 2
                rows = slice(half * HNPC, (half + 1) * HNPC)
                if is_t:
                    nc.gpsimd.collective_compute(
                        "AllGather",
                        mybir.AluOpType.bypass,
                        replica_groups=[list(range(C))],
                        ins=[kv_bnc[rows, :]],
                        outs=[kv_tabs[t][half][:]],
                    )
                else:
                    nc.gpsimd.collective_compute(
                        "AllGather",
                        mybir.AluOpType.bypass,
                        replica_groups=[list(range(C))],
                        ins=[gat_bnc[rows, :]],
                        outs=[gat_tabs[t][half][:]],
                    )

            def edge_pair(layer, bp):
                is_t = layer % 2 == 0
                t = layer // 2
                nb2 = min(2, NB - bp)
                NTa = NTTA if is_t else NTGA
                NTb = NTTB if is_t else NTGB
                NT = NTa + NTb
                TT = nb2 * NT
                N2a = nb2 * NTa
                islA = slice(bp * NTa * 8, (bp + nb2) * NTa * 8)
                islB = slice(bp * NTb * 8, (bp + nb2) * NTb * 8)
                islQ = slice(bp * NT * 8, (bp + nb2) * NT * 8)
                if is_t:
                    g_kv = gp.tile([P, 2 * NTMAX, 2 * HID], F8, tag="g_big")
                    nc.gpsimd.dma_gather(
                        g_kv[:, 0:N2a, :], kv_tabs[t][0][:], idxA_sb.ap()[:, islA],
                        N2a * P, N2a * P, 2 * HID, single_packet=False,
                    )
                    nc.gpsimd.dma_gather(
                        g_kv[:, N2a:TT, :], kv_tabs[t][1][:], idxB_sb.ap()[:, islB],
                        nb2 * NTb * P, nb2 * NTb * P, 2 * HID, single_packet=False,
                    )
                    g_q = gp.tile([P, 2 * NTMAX, HID], F8, tag="g_small")
                    nc.gpsimd.dma_gather(
                        g_q[:, 0:TT, :], q_tab[:], qad_sb.ap()[:, islQ],
                        TT * P, TT * P, HID, single_packet=False,
                    )
                    vpart = g_kv[:, 0:TT, HID : 2 * HID]
                    S_d = St_d
                else:
                    g_kv = gp.tile([P, 2 * NTMAX, GATW], F16, tag="g_big")
                    nc.gpsimd.dma_gather(
                        g_kv[:, 0:N2a, :], gat_tabs[t][0][:], idxA_sb.ap()[:, islA],
                        N2a * P, N2a * P, GATW, single_packet=False,
                    )
                    nc.gpsimd.dma_gather(
                        g_kv[:, N2a:TT, :], gat_tabs[t][1][:], idxB_sb.ap()[:, islB],
                        nb2 * NTb * P, nb2 * NTb * P, GATW, single_packet=False,
                    )
                    g_q = gp.tile([P, 2 * NTMAX, P], F16, tag="g_small")
                    nc.gpsimd.dma_gather(
                        g_q[:, 0:TT, :], ad_tab[:], qad_sb.ap()[:, islQ],
                        TT * P, TT * P, P, single_packet=False,
                    )
                    vpart = g_kv[:, 0:TT, 0 : HID // 2].bitcast(F8)
                    S_d = Sg_d

                rhs = gp.tile([P, 2 * NTMAX, HID + HEADS], F16, tag="rhs")
                red = gp.tile([P, 2 * NTMAX * HEADS], F32, tag="red")
                expdst = rhs[:, 0:TT, HID : HID + HEADS]
                if is_t:
                    nc.vector.tensor_tensor(
                        out=rhs[:, 0:TT, 0:HID],
                        in0=g_kv[:, 0:TT, 0:HID],
                        in1=g_q[:, 0:TT, :],
                        op=mybir.AluOpType.mult,
                    )
                    nc.vector.tensor_reduce(
                        out=red[:, 0 : TT * HEADS],
                        in_=rhs[:, 0:TT, 0:HID].rearrange(
                            "p t (h w) -> p t h w", h=HEADS
                        ),
                        axis=mybir.AxisListType.X,
                        op=mybir.AluOpType.add,
                    )
                    nc.scalar.activation(
                        expdst,
                        red[:, 0 : TT * HEADS].rearrange("p (t h) -> p t h", h=HEADS),
                        AF.Exp,
                        scale=SQ32,
                    )
                else:
                    esum = gp.tile([P, 2 * NTMAX * HEADS], F16, tag="esum")
                    nc.vector.tensor_tensor(
                        out=esum[:, 0 : TT * HEADS].rearrange(
                            "p (t h) -> p t h", h=HEADS
                        ),
                        in0=g_kv[:, 0:TT, HID // 2 : HID // 2 + HEADS],
                        in1=g_q[:, 0:TT, 0:HEADS],
                        op=mybir.AluOpType.add,
                    )
                    # leaky_relu(x, 0.2) = 0.6x + 0.4|x|
                    red_abs = gp.tile([P, 2 * NTMAX * HEADS], F16, tag="red_abs")
                    nc.scalar.activation(
                        red_abs[:, 0 : TT * HEADS],
                        esum[:, 0 : TT * HEADS],
                        AF.Abs,
                        scale=0.4,
                    )
                    nc.vector.scalar_tensor_tensor(
                        out=red[:, 0 : TT * HEADS],
                        in0=esum[:, 0 : TT * HEADS],
                        scalar=0.6,
                        in1=red_abs[:, 0 : TT * HEADS],
                        op0=mybir.AluOpType.mult,
                        op1=mybir.AluOpType.add,
                    )
                    nc.scalar.activation(
                        expdst,
                        red[:, 0 : TT * HEADS].rearrange("p (t h) -> p t h", h=HEADS),
                        AF.Exp,
                    )
                nc.vector.tensor_tensor(
                    out=rhs[:, 0:TT, 0:HID].rearrange("p t (h w) -> p t h w", h=HEADS),
                    in0=vpart.rearrange("p t (h w) -> p t h w", h=HEADS),
                    in1=expdst.to_broadcast([P, TT, HEADS, HDIM]),
                    op=mybir.AluOpType.mult,
                )
                # per-block scatter + skip matmuls
                aggs = []
                for i in range(nb2):
                    b = bp + i
                    S_sb = gp.tile([P, NTMAX * P], F8, tag="S_sb")
                    nc.sync.dma_start(
                        S_sb[:, 0 : NT * P], S_d.ap()[b * P : (b + 1) * P, :]
                    )
                    ps_agg = psA.tile([P, HID + HEADS], F32, space="PSUM", tag="ps_agg")
                    njs = 0
                    for jj in range(NTa):
                        nc.tensor.matmul(
                            ps_agg[:],
                            lhsT=S_sb[:, jj * P : (jj + 1) * P],
                            rhs=rhs[:, i * NTa + jj, :],
                            start=(njs == 0),
                            stop=False,
                        )
                        njs += 1
                    for jj in range(NTb):
                        nc.tensor.matmul(
                            ps_agg[:],
                            lhsT=S_sb[:, (NTa + jj) * P : (NTa + jj + 1) * P],
                            rhs=rhs[:, N2a + i * NTb + jj, :],
                            start=False,
                            stop=(jj == NTb - 1),
                        )
                    l0, l1 = hT_slices(b)
                    ps_skip = psB.tile([P, 2 * HID], F32, space="PSUM", tag="ps_b")
                    if is_t:
                        mm_dense(
                            ps_skip[:, 0:HID], l0, l1, wt[f"wsk{t}"], HID,
                            wt[f"bsk{t}"],
                        )
                    else:
                        nc.tensor.matmul(
                            ps_skip[:, 0:HID], lhsT=ones1[:], rhs=wt[f"bg{t}"][:],
                            start=True, stop=True,
                        )
                    aggs.append((ps_agg, ps_skip))

                # normalize + LN + relu, batched over the pair
                t2p = sp.tile([P, 2, HID], F32, tag="t2p")
                for i in range(nb2):
                    ps_agg, ps_skip = aggs[i]
                    den = sp.tile([P, HEADS], F32, tag="den")
                    nc.vector.tensor_scalar(
                        out=den[:], in0=ps_agg[:, HID : HID + HEADS],
                        scalar1=1e-16, scalar2=None, op0=mybir.AluOpType.add,
                    )
                    rec = sp.tile([P, HEADS], F32, tag="rec")
                    nc.vector.reciprocal(rec[:], den[:])
                    t1 = sp.tile([P, HID], F32, tag="t1")
                    nc.vector.tensor_tensor(
                        out=t1[:].rearrange("p (h w) -> p h w", h=HEADS),
                        in0=ps_agg[:, 0:HID].rearrange("p (h w) -> p h w", h=HEADS),
                        in1=rec[:].to_broadcast([P, HEADS, HDIM]),
                        op=mybir.AluOpType.mult,
                    )
                    nc.vector.tensor_tensor(
                        out=t2p[:, i, :], in0=t1[:], in1=ps_skip[:, 0:HID],
                        op=mybir.AluOpType.add,
                    )
                t2v = t2p[:, 0:nb2, :]
                if not is_t:
                    nc.vector.tensor_tensor(
                        out=t2v, in0=t2v,
                        in1=h_all.ap()[:, bp * HID : (bp + nb2) * HID].rearrange(
                            "p (i f) -> p i f", i=nb2
                        ),
                        op=mybir.AluOpType.add,
                    )
                mu = sp.tile([P, 2], F32, tag="mu")
                nc.vector.tensor_reduce(
                    out=mu[:, 0:nb2], in_=t2v, axis=mybir.AxisListType.X,
                    op=mybir.AluOpType.add,
                )
                nc.vector.tensor_scalar(
                    out=mu[:, 0:nb2], in0=mu[:, 0:nb2], scalar1=1.0 / HID,
                    scalar2=None, op0=mybir.AluOpType.mult,
                )
                nc.vector.tensor_tensor(
                    out=t2v, in0=t2v,
                    in1=mu[:, 0:nb2].to_broadcast([P, nb2, HID]),
                    op=mybir.AluOpType.subtract,
                )
                sq = sp.tile([P, 2, HID], F32, tag="sq")
                nc.scalar.activation(sq[:, 0:nb2, :], t2v, AF.Square)
                s2 = sp.tile([P, 2], F32, tag="s2")
                nc.vector.tensor_reduce(
                    out=s2[:, 0:nb2], in_=sq[:, 0:nb2, :],
                    axis=mybir.AxisListType.X, op=mybir.AluOpType.add,
                )
                sd = sp.tile([P, 2], F32, tag="sd")
                nc.scalar.activation(
                    sd[:, 0:nb2], s2[:, 0:nb2], AF.Sqrt, scale=1.0 / HID,
                    bias=eps_t[:, 0:1],
                )
                rs = sp.tile([P, 2], F32, tag="rs")
                nc.vector.reciprocal(rs[:, 0:nb2], sd[:, 0:nb2])
                nc.vector.tensor_tensor(
                    out=t2v, in0=t2v,
                    in1=rs[:, 0:nb2].to_broadcast([P, nb2, HID]),
                    op=mybir.AluOpType.mult,
                )
                if not ln_trivial:
                    nc.vector.tensor_tensor(
                        out=t2v, in0=t2v,
                        in1=wt[f"lng{layer}"][:]
                        .rearrange("p (o f) -> p o f", o=1)
                        .to_broadcast([P, nb2, HID]),
                        op=mybir.AluOpType.mult,
                    )
                    nc.vector.tensor_tensor(
                        out=t2v, in0=t2v,
                        in1=wt[f"lnb{layer}"][:]
                        .rearrange("p (o f) -> p o f", o=1)
                        .to_broadcast([P, nb2, HID]),
                        op=mybir.AluOpType.add,
                    )
                hdst = h_all.ap()[:, bp * HID : (bp + nb2) * HID]
                nc.vector.tensor_scalar(
                    out=hdst.rearrange("p (i f) -> p i f", i=nb2), in0=t2v,
                    scalar1=0.0, scalar2=None, op0=mybir.AluOpType.max,
                )
                for i in range(nb2):
                    b = bp + i
                    store_hT(b)
                    if layer == 3:
                        nc.tensor.matmul(
                            psum_pool[:],
                            lhsT=Sp_sb[b][:],
                            rhs=h_all.ap()[:, b * HID : (b + 1) * HID],
                            start=(b == 0),
                            stop=(b == NB - 1),
                        )

            # layer 0 dense phase + its chunked AllGathers
            load_layer_idx(True)
            nc.sync.dma_start(q_tab[LOCN - 1 : LOCN, :], zrow8_t[:, 0:HID])
            nc.sync.dma_start(ad_tab[LOCN - 1 : LOCN, :], zrow_t[:, 0:P])
            for bp in range(0, NB, 2):
                dense_pair(0, bp)
                if bp == NB // 2 - 2:
                    emit_ag(0, 0)
            emit_ag(0, 1)

            # main loop: edge phase of layer L interleaved with dense phase of
            # layer L+1; each half-AllGather is emitted as soon as its bounce
            # rows are complete so it overlaps remaining edge/dense work.
            for layer in range(4):
                for bp in range(0, NB, 2):
                    edge_pair(layer, bp)
                    if layer < 3:
                        dense_pair(layer + 1, bp)
                        if bp == NB // 2 - 2:
                            emit_ag(layer + 1, 0)
                        elif bp == NB - 2:
                            emit_ag(layer + 1, 1)
                if layer < 3:
                    load_layer_idx((layer + 1) % 2 == 0)  # for the next layer's type

            # ---- pool + MLP ----
            pool_sb = sp.tile([N_GRAPHS, HID], F32, tag="pool_sb")
            nc.scalar.activation(pool_sb[:], psum_pool[:], AF.Copy)
            nc.sync.dma_start(pool_in[:], pool_sb[:])
            nc.gpsimd.collective_compute(
                "AllReduce",
                mybir.AluOpType.add,
                replica_groups=[list(range(C))],
                ins=[pool_in.opt()],
                outs=[pool_out.opt()],
            )
            sums = sp.tile([N_GRAPHS, HID], F32, tag="sums")
            nc.sync.dma_start(sums[:], pool_out[:])
            pooled = sp.tile([N_GRAPHS, HID], F32, tag="pooled")
            nc.vector.tensor_scalar(
                out=pooled[:], in0=sums[:], scalar1=invcnt_t[:, 0:1],
                scalar2=None, op0=mybir.AluOpType.mult,
            )
            p16 = sp.tile([N_GRAPHS, HID], F16, tag="p16")
            nc.scalar.activation(p16[:], pooled[:], AF.Copy)
            pT = sp.tile([P, 2 * N_GRAPHS], F16, tag="pT")
            for f in range(2):
                ptp = psT.tile([P, P], F16, space="PSUM", tag="ptp")
                nc.tensor.transpose(ptp[:], p16[:, f * P : (f + 1) * P], ident[:])
                nc.scalar.activation(
                    pT[:, f * N_GRAPHS : (f + 1) * N_GRAPHS], ptp[:], AF.Copy
                )
            ps1 = psA.tile([P, 2 * HID], F32, space="PSUM", tag="ps_dense")
            nc.tensor.matmul(
                ps1[:], lhsT=pT[:, 0:N_GRAPHS], rhs=w1_t[:, 0 : 2 * HID],
                start=True, stop=False,
            )
            nc.tensor.matmul(
                ps1[:], lhsT=pT[:, N_GRAPHS : 2 * N_GRAPHS],
                rhs=w1_t[:, 2 * HID : 4 * HID], start=False, stop=False,
            )
            nc.tensor.matmul(
                ps1[:], lhsT=ones1[:], rhs=b1_t[:], start=False, stop=True
            )
            h1 = sp.tile([N_GRAPHS, 2 * HID], F16, tag="h1")
            nc.scalar.activation(h1[:], ps1[:], AF.Relu)
            h1T = sp.tile([P, 4 * N_GRAPHS], F16, tag="h1T")
            for f in range(4):
                ptp = psT.tile([P, P], F16, space="PSUM", tag="ptp")
                nc.tensor.transpose(ptp[:], h1[:, f * P : (f + 1) * P], ident[:])
                nc.scalar.activation(
                    h1T[:, f * N_GRAPHS : (f + 1) * N_GRAPHS], ptp[:], AF.Copy
                )
            ps2 = psB.tile([P, 2 * HID], F32, space="PSUM", tag="ps_b")
            for f in range(4):
                nc.tensor.matmul(
                    ps2[:, 0:OUT_DIM],
                    lhsT=h1T[:, f * N_GRAPHS : (f + 1) * N_GRAPHS],
                    rhs=w2_t[:, f * OUT_DIM : (f + 1) * OUT_DIM],
                    start=(f == 0),
                    stop=False,
                )
            nc.tensor.matmul(
                ps2[:, 0:OUT_DIM], lhsT=ones1[:], rhs=b2_t[:], start=False, stop=True
            )
            out_sb = sp.tile([N_GRAPHS, OUT_DIM], F32, tag="out_sb")
            nc.scalar.activation(out_sb[:], ps2[:, 0:OUT_DIM], AF.Copy)
            nc.sync.dma_start(out_d.ap(), out_sb[:])

    nc.compile()
    return nc


_CACHE = {}


def kernel(**inputs):
    meta, in_maps = host_prep(inputs)
    key = tuple(sorted(meta.items()))
    if key not in _CACHE:
        _CACHE[key] = build_program(meta)
    nc = _CACHE[key]
    res = bass_utils.run_bass_kernel_spmd(nc, in_maps, core_ids=list(range(C)))
    return np.asarray(res.results[0]["out"], np.float32)

